# revision 1
# baseline (speedup 1.0000x reference)
"""JaccardLoss kernel for Trainium2 (8 NeuronCores, Bass/Tile).

Contract: kernel(output, target) takes the FULL [32, 1, 1024, 1024] f32
inputs (values exactly 0.0/1.0) and returns the scalar f32 loss:
  per (b, c) slice: inter = sum(o==1 & t==1), union = sum(o==1 | t==1),
  iou = inter / (union + 1e-7); result = mean(iou) * 100.

Strategy (data-parallel): shard B=32 across the 8 cores (4 slices per
core). Each core computes per-slice partial sums (sum o, sum t,
sum o*t) on device; the final 32-element iou/mean math runs on host.
For 0/1 data: inter = sum(o*t), union = sum(o) + sum(t) - inter.

Per-core kernel layout: the 4 slices are viewed as one [128, 32768] f32
tensor (slice s = partitions 32s..32s+31, each partition row contiguous
in HBM). DMA streams [128, 2048] chunks; VectorE forms the elementwise
product; ScalarE reduces the product along the free dim (activation
Copy with accum_out); TensorE reduces o and t across partitions per
slice via matmuls against a constant slice-indicator matrix E [128, 4]
in float32r (full-rate fp32 streaming, exact for 0/1 values), with
PSUM accumulation across all 64 [128,512] sub-chunks. The kernel is
memory-bound: ~32 MiB HBM reads per core, ~93 us measured
(~360 GB/s/core effective HBM bandwidth).
"""

import numpy as np

import concourse.bacc as bacc
import concourse.tile as tile
from concourse import mybir
from concourse.bass_utils import run_bass_kernel_spmd

N_CORES = 8
P = 128
NSLICE = 4  # batch slices per core
W = 32768  # free width of the per-core [128, W] view
CHUNK = 2048
SUB = 512  # matmul moving free dim (one PSUM bank of f32)
EPS = np.float32(1e-07)

_nc_cache = None


def _build_nc():
    nch = W // CHUNK
    nsub = CHUNK // SUB
    ntot = nch * nsub

    nc = bacc.Bacc("TRN2", target_bir_lowering=False, debug=False)
    f32 = mybir.dt.float32
    f32r = mybir.dt.float32r

    o_d = nc.dram_tensor("o", [P, W], f32r, kind="ExternalInput")
    t_d = nc.dram_tensor("t", [P, W], f32r, kind="ExternalInput")
    e_d = nc.dram_tensor("emat", [P, NSLICE], f32r, kind="ExternalInput")
    r_d = nc.dram_tensor("res", [NSLICE, 3], f32, kind="ExternalOutput")

    with (
        tile.TileContext(nc) as tc,
        tc.tile_pool(name="singles", bufs=1) as singles,
        tc.tile_pool(name="io", bufs=6) as io,
        tc.tile_pool(name="prod", bufs=3) as prod,
        tc.tile_pool(name="small", bufs=2) as small,
        tc.tile_pool(name="psum", bufs=2, space="PSUM") as psum,
        tc.tile_pool(name="psum_sm", bufs=2, space="PSUM") as psum_sm,
    ):
        e_tile = singles.tile([P, NSLICE], f32r)
        nc.sync.dma_start(out=e_tile[:], in_=e_d[:])

        acc = small.tile([P, nch], f32, tag="acc")
        ps_o = psum.tile([NSLICE, SUB], f32, space="PSUM", tag="ps_o")
        ps_t = psum.tile([NSLICE, SUB], f32, space="PSUM", tag="ps_t")

        for c in range(nch):
            o_tile = io.tile([P, CHUNK], f32r, tag="o")
            t_tile = io.tile([P, CHUNK], f32r, tag="t")
            nc.sync.dma_start(out=o_tile[:], in_=o_d[:, c * CHUNK : (c + 1) * CHUNK])
            nc.sync.dma_start(out=t_tile[:], in_=t_d[:, c * CHUNK : (c + 1) * CHUNK])

            p_tile = prod.tile([P, CHUNK], f32, tag="p")
            nc.vector.tensor_mul(
                p_tile[:], o_tile[:].bitcast(f32), t_tile[:].bitcast(f32)
            )
            scr_tile = prod.tile([P, CHUNK], f32, tag="scr")
            nc.scalar.activation(
                out=scr_tile[:],
                in_=p_tile[:],
                func=mybir.ActivationFunctionType.Copy,
                accum_out=acc[:, c : c + 1],
            )

            for s in range(nsub):
                k = c * nsub + s
                st = k == 0
                sp = k == ntot - 1
                rhs_o = o_tile[:, s * SUB : (s + 1) * SUB]
                rhs_t = t_tile[:, s * SUB : (s + 1) * SUB]
                nc.tensor.matmul(ps_o[:], e_tile[:], rhs_o, start=st, stop=sp)
                nc.tensor.matmul(ps_t[:], e_tile[:], rhs_t, start=st, stop=sp)

        res_tile = small.tile([NSLICE, 3], f32, tag="res")
        nc.vector.reduce_sum(res_tile[:, 0:1], ps_o[:], axis=mybir.AxisListType.X)
        nc.vector.reduce_sum(res_tile[:, 1:2], ps_t[:], axis=mybir.AxisListType.X)

        icol = small.tile([P, 1], f32, tag="icol")
        nc.vector.reduce_sum(icol[:], acc[:], axis=mybir.AxisListType.X)
        ps_i = psum_sm.tile([NSLICE, 1], f32, space="PSUM", tag="ps_i")
        nc.tensor.matmul(
            ps_i[:], e_tile[:].bitcast(f32), icol[:], start=True, stop=True
        )
        nc.vector.tensor_copy(res_tile[:, 2:3], ps_i[:])

        nc.sync.dma_start(out=r_d[:], in_=res_tile[:])

    nc.compile()
    return nc


def kernel(output, target):
    global _nc_cache
    if _nc_cache is None:
        _nc_cache = _build_nc()
    nc = _nc_cache

    o = np.ascontiguousarray(np.asarray(output, dtype=np.float32)).reshape(32, -1)
    t = np.ascontiguousarray(np.asarray(target, dtype=np.float32)).reshape(32, -1)
    emat = np.zeros((P, NSLICE), np.float32)
    emat[np.arange(P), np.arange(P) // 32] = 1.0

    in_maps = [
        {
            "o": o[NSLICE * c : NSLICE * (c + 1)].reshape(P, W),
            "t": t[NSLICE * c : NSLICE * (c + 1)].reshape(P, W),
            "emat": emat,
        }
        for c in range(N_CORES)
    ]

    last_err = None
    for _ in range(3):  # the axon tunnel occasionally drops a dispatch
        try:
            results = run_bass_kernel_spmd(nc, in_maps, list(range(N_CORES))).results
            break
        except Exception as e:  # noqa: BLE001
            last_err = e
    else:
        raise last_err

    res = np.stack([r["res"] for r in results]).reshape(32, 3).astype(np.float32)
    sum_o, sum_t, inter = res[:, 0], res[:, 1], res[:, 2]
    union = sum_o + sum_t - inter
    ious = inter / (union + EPS)
    return (np.mean(ious) * np.float32(100.0)).astype(np.float32)



# revision 2
# speedup vs baseline: 1.0127x; 1.0127x over previous
"""JaccardLoss kernel for Trainium2 (8 NeuronCores, Bass/Tile).

Contract: kernel(output, target) takes the FULL [32, 1, 1024, 1024] f32
inputs (values exactly 0.0/1.0) and returns the scalar f32 loss:
  per (b, c) slice: inter = sum(o==1 & t==1), union = sum(o==1 | t==1),
  iou = inter / (union + 1e-7); result = mean(iou) * 100.

Strategy (data-parallel): shard B=32 across the 8 cores (4 slices per
core, viewed as one [128, 32768] f32 tensor per core; slice s =
partitions 32s..32s+31). For 0/1 data:
  inter = sum(o*t), union = sum(o) + sum(t) - inter,
so each core only needs per-slice [sum_o, sum_t, inter]; the final
32-element iou/mean math runs on host.

Per 2048-column chunk the kernel runs one full pass over the data on
each of three engines, so every engine stays below the DMA cadence:
  SP  : two 1 MiB HBM->SBUF DMAs (o, t) on the SP HWDGE ring
  DVE : one fused scalar_tensor_tensor (o*t) with accum_out
        -> per-partition inter column
  ACT : two activation(Copy) ops with accum_out
        -> per-partition sum_o / sum_t columns
(no per-chunk matmuls; nc.vector.tensor_tensor_reduce is avoided — it
crashes this runtime). The tail reduces the per-chunk columns to
[128, 3], multiplies by a constant slice-indicator matrix E [128, 4] on
TensorE to get per-slice sums, and DMAs the [4, 3] result out on ACT's
HWDGE ring so the SP chunk-DMA FIFO is never stalled by the tail.

The kernel is memory-bound: 32 MiB HBM reads per core at the ~368 GB/s
per-core streaming rate = 91 us floor; measured steady-state is ~96 us
per pass (loop-difference method over an in-NEFF For_i loop).
"""

import numpy as np

import concourse.bacc as bacc
import concourse.tile as tile
from concourse import mybir
from concourse.bass_utils import run_bass_kernel_spmd

N_CORES = 8
P = 128
NSLICE = 4  # batch slices per core
W = 32768  # free width of the per-core [128, W] view
CHUNK = 2048
IO_BUFS = 8
EPS = np.float32(1e-07)

_nc_cache = None


def _build_nc():
    nch = W // CHUNK

    nc = bacc.Bacc("TRN2", target_bir_lowering=False, debug=False)
    f32 = mybir.dt.float32
    mult = mybir.AluOpType.mult

    o_d = nc.dram_tensor("o", [P, W], f32, kind="ExternalInput")
    t_d = nc.dram_tensor("t", [P, W], f32, kind="ExternalInput")
    e_d = nc.dram_tensor("emat", [P, NSLICE], f32, kind="ExternalInput")
    r_d = nc.dram_tensor("res", [NSLICE, 3], f32, kind="ExternalOutput")

    with (
        tile.TileContext(nc) as tc,
        tc.tile_pool(name="singles", bufs=1) as singles,
        tc.tile_pool(name="io", bufs=IO_BUFS) as io,
        tc.tile_pool(name="scr", bufs=2) as scr,
        tc.tile_pool(name="small", bufs=2) as small,
        tc.tile_pool(name="psum_sm", bufs=2, space="PSUM") as psum_sm,
    ):
        e_tile = singles.tile([P, NSLICE], f32)
        nc.sync.dma_start(out=e_tile[:], in_=e_d[:])

        acc_o = small.tile([P, nch], f32, tag="acc_o")
        acc_t = small.tile([P, nch], f32, tag="acc_t")
        acc_i = small.tile([P, nch], f32, tag="acc_i")

        for c in range(nch):
            o_tile = io.tile([P, CHUNK], f32, tag="o")
            t_tile = io.tile([P, CHUNK], f32, tag="t")
            nc.sync.dma_start(out=o_tile[:], in_=o_d[:, c * CHUNK : (c + 1) * CHUNK])
            nc.sync.dma_start(out=t_tile[:], in_=t_d[:, c * CHUNK : (c + 1) * CHUNK])

            s_i = scr.tile([P, CHUNK], f32, tag="scr_i")
            nc.vector.scalar_tensor_tensor(
                out=s_i[:],
                in0=o_tile[:],
                scalar=1.0,
                in1=t_tile[:],
                op0=mult,
                op1=mult,
                accum_out=acc_i[:, c : c + 1],
            )
            s_o = scr.tile([P, CHUNK], f32, tag="scr_o")
            nc.scalar.activation(
                out=s_o[:],
                in_=o_tile[:],
                func=mybir.ActivationFunctionType.Copy,
                accum_out=acc_o[:, c : c + 1],
            )
            s_t = scr.tile([P, CHUNK], f32, tag="scr_t")
            nc.scalar.activation(
                out=s_t[:],
                in_=t_tile[:],
                func=mybir.ActivationFunctionType.Copy,
                accum_out=acc_t[:, c : c + 1],
            )

        icols = small.tile([P, 3], f32, tag="icols")
        nc.vector.reduce_sum(icols[:, 0:1], acc_o[:], axis=mybir.AxisListType.X)
        nc.vector.reduce_sum(icols[:, 1:2], acc_t[:], axis=mybir.AxisListType.X)
        nc.vector.reduce_sum(icols[:, 2:3], acc_i[:], axis=mybir.AxisListType.X)

        ps = psum_sm.tile([NSLICE, 3], f32, space="PSUM", tag="ps")
        nc.tensor.matmul(ps[:], e_tile[:], icols[:], start=True, stop=True)

        res_tile = small.tile([NSLICE, 3], f32, tag="res")
        nc.vector.tensor_copy(res_tile[:], ps[:])
        nc.scalar.dma_start(out=r_d[:], in_=res_tile[:])

    nc.compile()
    return nc


def kernel(output, target):
    global _nc_cache
    if _nc_cache is None:
        _nc_cache = _build_nc()
    nc = _nc_cache

    o = np.ascontiguousarray(np.asarray(output, dtype=np.float32)).reshape(32, -1)
    t = np.ascontiguousarray(np.asarray(target, dtype=np.float32)).reshape(32, -1)
    emat = np.zeros((P, NSLICE), np.float32)
    emat[np.arange(P), np.arange(P) // 32] = 1.0

    in_maps = [
        {
            "o": o[NSLICE * c : NSLICE * (c + 1)].reshape(P, W),
            "t": t[NSLICE * c : NSLICE * (c + 1)].reshape(P, W),
            "emat": emat,
        }
        for c in range(N_CORES)
    ]

    last_err = None
    for _ in range(3):  # the axon tunnel occasionally drops a dispatch
        try:
            results = run_bass_kernel_spmd(nc, in_maps, list(range(N_CORES))).results
            break
        except Exception as e:  # noqa: BLE001
            last_err = e
    else:
        raise last_err

    res = np.stack([r["res"] for r in results]).reshape(32, 3).astype(np.float32)
    sum_o, sum_t, inter = res[:, 0], res[:, 1], res[:, 2]
    union = sum_o + sum_t - inter
    ious = inter / (union + EPS)
    return (np.mean(ious) * np.float32(100.0)).astype(np.float32)
